# revision 7
# baseline (speedup 1.0000x reference)
"""Multi-head attention + residual + LayerNorm Trainium2 kernel.

Problem: B=4, S=2048, D=2048, H=16 heads (dk=128), fp32 in/out.
  Q/K/V projections -> per-head softmax(QK^T/sqrt(dk)) -> context -> Wo
  -> +residual -> LayerNorm.  Returns (output, attn_probs).

Sharding: 8 cores = 4 batches x 2 query-halves.  Each core computes K/V
projections for its whole batch (duplicated across the pair), Q projection
and attention for its 1024 query rows.

The reference reinterprets the head-major (H*B, S, DK) context tensor as
(B, H, S, DK) before the output projection, which mixes context slices
across batches: output batch b' at feature slot h' uses the context of
(head 4*b' + h'//4, batch h'%4).  That makes the output projection
non-local under batch sharding, so the kernel runs as TWO NEFF launches:
  A: projections + attention  -> attn probs + normalized context^T
  B: output projection + residual + LayerNorm
with the host permuting the (8 MB/core) context slices between launches.

All matmuls run as float32r (TF32-like, ~11-bit mantissa, full PE rate at
free-dim >= 256) with fp32 PSUM accumulation.  Both score orientations are
computed on the TensorE (q-major for the attn output + softmax row sums via
the ScalarE's fused accum_out; k-major for the context matmul), avoiding all
transposes of the big matrices.
"""

import numpy as np

B, S, D, H = 4, 2048, 2048, 16
DK = 128
R = S // 2          # query rows per core
NDC = D // 128      # contraction chunks
EPS = 1e-6
ISQ = 1.0 / float(np.sqrt(DK))

_COMPILED = None
LAST_EXEC_NS = None
TRACE = False


def _build_a():
    import concourse.bass as bass
    from concourse import bacc, mybir
    import concourse.tile as tile
    from concourse.masks import make_identity

    fp32 = mybir.dt.float32
    fp32r = mybir.dt.float32r
    AF = mybir.ActivationFunctionType

    nc = bacc.Bacc("TRN2", target_bir_lowering=False, debug=False, num_devices=8)

    qt = nc.declare_dram_parameter("qt", [D, R], fp32r, isOutput=False)
    kt = nc.declare_dram_parameter("kt", [D, S], fp32r, isOutput=False)
    vt = nc.declare_dram_parameter("vt", [D, S], fp32r, isOutput=False)
    wqT = nc.declare_dram_parameter("wqT", [D, D], fp32r, isOutput=False)
    wkT = nc.declare_dram_parameter("wkT", [D, D], fp32r, isOutput=False)
    wvT = nc.declare_dram_parameter("wvT", [D, D], fp32r, isOutput=False)
    ones1 = nc.declare_dram_parameter("ones1", [1, 128], fp32r, isOutput=False)
    attn_o = nc.declare_dram_parameter("attn_o", [H, R, S], fp32, isOutput=True)
    ctx_s = nc.declare_dram_parameter("ctx_s", [H, DK, R], fp32r, isOutput=True)

    kt_s = nc.dram_tensor("kt_s", [D, S], fp32r)      # K^T, o-major
    v_s = nc.dram_tensor("v_s", [S, D], fp32r)        # V, k-major

    with tile.TileContext(nc) as tc:
        with (
            tc.tile_pool(name="persist", bufs=1) as persist,
        ):
            ident = persist.tile([128, 128], fp32)
            make_identity(nc, ident)
            ones_sb = persist.tile([1, 128], fp32r)
            nc.sync.dma_start(out=ones_sb, in_=ones1[:])

            # ---------------- Phase 1a: K^T projection ----------------
            with (
                tc.tile_pool(name="ktin", bufs=1) as ktin_pool,
                tc.tile_pool(name="wblk", bufs=2) as wblk_pool,
                tc.tile_pool(name="pev", bufs=3) as pev_pool,
                tc.tile_pool(name="pp", bufs=8, space="PSUM") as pp_pool,
            ):
                kt_sb = ktin_pool.tile([128, NDC, S], fp32r)
                nc.sync.dma_start(
                    out=kt_sb, in_=kt.rearrange("(c p) k -> p c k", p=128)
                )
                for ot in range(16):
                    wblk = wblk_pool.tile([128, NDC, 128], fp32r, tag="wblk")
                    nc.sync.dma_start(
                        out=wblk,
                        in_=wkT[:, ot * 128 : (ot + 1) * 128].rearrange(
                            "(c p) o -> p c o", p=128
                        ),
                    )
                    for kc4 in range(4):
                        ps = pp_pool.tile([128, 512], fp32, tag="pp")
                        for dc in range(NDC):
                            nc.tensor.matmul(
                                ps,
                                wblk[:, dc, :],
                                kt_sb[:, dc, kc4 * 512 : (kc4 + 1) * 512],
                                start=(dc == 0),
                                stop=(dc == NDC - 1),
                            )
                        ev = pev_pool.tile([128, 512], fp32r, tag="pev")
                        nc.scalar.copy(ev, ps)
                        nc.sync.dma_start(
                            out=kt_s[
                                ot * 128 : (ot + 1) * 128,
                                kc4 * 512 : (kc4 + 1) * 512,
                            ],
                            in_=ev,
                        )

            # ---------------- Phase 1b: V projection ----------------
            with (
                tc.tile_pool(name="wvres", bufs=1) as wvres_pool,
                tc.tile_pool(name="vtin", bufs=3) as vtin_pool,
                tc.tile_pool(name="pev2", bufs=3) as pev2_pool,
                tc.tile_pool(name="pp2", bufs=8, space="PSUM") as pp2_pool,
            ):
                wv_sb = wvres_pool.tile([128, NDC, D], fp32r)
                nc.sync.dma_start(
                    out=wv_sb, in_=wvT.rearrange("(c p) o -> p c o", p=128)
                )
                for rt in range(16):
                    vtblk = vtin_pool.tile([128, NDC, 128], fp32r, tag="vtblk")
                    nc.sync.dma_start(
                        out=vtblk,
                        in_=vt[:, rt * 128 : (rt + 1) * 128].rearrange(
                            "(c p) r -> p c r", p=128
                        ),
                    )
                    for og in range(4):
                        ps = pp2_pool.tile([128, 512], fp32, tag="pp2")
                        for dc in range(NDC):
                            nc.tensor.matmul(
                                ps,
                                vtblk[:, dc, :],
                                wv_sb[:, dc, og * 512 : (og + 1) * 512],
                                start=(dc == 0),
                                stop=(dc == NDC - 1),
                            )
                        ev = pev2_pool.tile([128, 512], fp32r, tag="pev2")
                        nc.scalar.copy(ev, ps)
                        nc.sync.dma_start(
                            out=v_s[
                                rt * 128 : (rt + 1) * 128,
                                og * 512 : (og + 1) * 512,
                            ],
                            in_=ev,
                        )

            # ---------------- Phase 1c: Q^T projection (stays in SBUF) ----
            with tc.tile_pool(name="qtres", bufs=1) as qtres_pool:
                QT = qtres_pool.tile([128, H, R], fp32r)
                with (
                    tc.tile_pool(name="qtin", bufs=1) as qtin_pool,
                    tc.tile_pool(name="wblk3", bufs=2) as wblk3_pool,
                    tc.tile_pool(name="pp3", bufs=8, space="PSUM") as pp3_pool,
                ):
                    qt_sb = qtin_pool.tile([128, NDC, R], fp32r)
                    nc.sync.dma_start(
                        out=qt_sb, in_=qt.rearrange("(c p) r -> p c r", p=128)
                    )
                    for ot in range(16):
                        wblk = wblk3_pool.tile([128, NDC, 128], fp32r, tag="wblk3")
                        nc.sync.dma_start(
                            out=wblk,
                            in_=wqT[:, ot * 128 : (ot + 1) * 128].rearrange(
                                "(c p) o -> p c o", p=128
                            ),
                        )
                        for rc in range(2):
                            ps = pp3_pool.tile([128, 512], fp32, tag="pp3")
                            for dc in range(NDC):
                                nc.tensor.matmul(
                                    ps,
                                    wblk[:, dc, :],
                                    qt_sb[:, dc, rc * 512 : (rc + 1) * 512],
                                    start=(dc == 0),
                                    stop=(dc == NDC - 1),
                                )
                            nc.vector.tensor_copy(
                                QT[:, ot, rc * 512 : (rc + 1) * 512], ps
                            )

                # ---------------- Phase 2: attention per head ----------------
                with (
                    tc.tile_pool(name="kvh", bufs=2) as kvh_pool,
                    tc.tile_pool(name="aq", bufs=3) as aq_pool,
                    tc.tile_pool(name="expt", bufs=3) as expt_pool,
                    tc.tile_pool(name="zs", bufs=2) as zs_pool,
                    tc.tile_pool(name="ctxe", bufs=2) as ctxe_pool,
                    tc.tile_pool(name="psA", bufs=3, space="PSUM") as psA_pool,
                    tc.tile_pool(name="psC", bufs=1, space="PSUM") as psC_pool,
                ):
                    for h in range(H):
                        kth = kvh_pool.tile([128, S], fp32r, tag="kth")
                        nc.sync.dma_start(
                            out=kth, in_=kt_s[h * 128 : (h + 1) * 128, :]
                        )
                        vh = kvh_pool.tile([128, 16, 128], fp32r, tag="vh")
                        nc.sync.dma_start(
                            out=vh,
                            in_=v_s[:, h * 128 : (h + 1) * 128].rearrange(
                                "(kc p) d -> p kc d", p=128
                            ),
                        )

                        # ---- q-major: attn output + row sums ----
                        z2 = zs_pool.tile([128, 16], fp32, tag="z2")
                        iz = zs_pool.tile([128, 8], fp32, tag="iz")
                        for qt_i in range(8):
                            aq = aq_pool.tile([128, S], fp32, tag="aq")
                            for half in range(2):
                                sq = psA_pool.tile([128, 1024], fp32, tag="scA")
                                for kg in range(2):
                                    kk = half * 1024 + kg * 512
                                    nc.tensor.matmul(
                                        sq[:, kg * 512 : (kg + 1) * 512],
                                        QT[:, h, qt_i * 128 : (qt_i + 1) * 128],
                                        kth[:, kk : kk + 512],
                                        start=True,
                                        stop=True,
                                    )
                                nc.scalar.activation(
                                    aq[:, half * 1024 : (half + 1) * 1024],
                                    sq,
                                    AF.Exp,
                                    scale=ISQ,
                                    accum_out=z2[:, qt_i * 2 + half : qt_i * 2 + half + 1],
                                )
                            nc.vector.tensor_add(
                                iz[:, qt_i : qt_i + 1],
                                z2[:, qt_i * 2 : qt_i * 2 + 1],
                                z2[:, qt_i * 2 + 1 : qt_i * 2 + 2],
                            )
                            nc.vector.reciprocal(
                                iz[:, qt_i : qt_i + 1], iz[:, qt_i : qt_i + 1]
                            )
                            nc.vector.tensor_scalar_mul(
                                aq, in0=aq, scalar1=iz[:, qt_i : qt_i + 1]
                            )
                            nc.sync.dma_start(
                                out=attn_o[h, qt_i * 128 : (qt_i + 1) * 128, :],
                                in_=aq,
                            )

                        # iz [128q, 8qt] -> izflat [1, 1024] (per-column PE
                        # transposes) -> broadcast to [128, 1024] via an
                        # outer product with a ones column (K=1 matmul).
                        izt_ps = psA_pool.tile([1, 1024], fp32, tag="scA")
                        for qt_i in range(8):
                            nc.tensor.transpose(
                                izt_ps[:, qt_i * 128 : (qt_i + 1) * 128],
                                iz[:, qt_i : qt_i + 1],
                                ident,
                            )
                        izflat = zs_pool.tile([1, 1024], fp32r, tag="izflat")
                        nc.vector.tensor_copy(izflat, izt_ps)
                        izrep_ps = psA_pool.tile([128, 1024], fp32, tag="scA")
                        for qg in range(2):
                            nc.tensor.matmul(
                                izrep_ps[:, qg * 512 : (qg + 1) * 512],
                                ones_sb,
                                izflat[:, qg * 512 : (qg + 1) * 512],
                                start=True,
                                stop=True,
                            )
                        izrep = zs_pool.tile([128, 1024], fp32, tag="izrep")
                        nc.vector.tensor_copy(izrep, izrep_ps)

                        # ---- k-major: exp(scores^T) and context ----
                        ctx_ps = psC_pool.tile([128, 1024], fp32, tag="ctx")
                        for kc in range(16):
                            st = psA_pool.tile([128, 1024], fp32, tag="scA")
                            for qg in range(2):
                                nc.tensor.matmul(
                                    st[:, qg * 512 : (qg + 1) * 512],
                                    kth[:, kc * 128 : (kc + 1) * 128],
                                    QT[:, h, qg * 512 : (qg + 1) * 512],
                                    start=True,
                                    stop=True,
                                )
                            et = expt_pool.tile([128, 1024], fp32r, tag="expt")
                            nc.scalar.activation(et, st, AF.Exp, scale=ISQ)
                            for qg in range(2):
                                nc.tensor.matmul(
                                    ctx_ps[:, qg * 512 : (qg + 1) * 512],
                                    vh[:, kc, :],
                                    et[:, qg * 512 : (qg + 1) * 512],
                                    start=(kc == 0),
                                    stop=(kc == 15),
                                )
                        ctx_sb = ctxe_pool.tile([128, 1024], fp32r, tag="ctxsb")
                        nc.vector.tensor_mul(ctx_sb, ctx_ps, izrep)
                        nc.sync.dma_start(out=ctx_s[h], in_=ctx_sb)

    nc.compile()
    return nc


def _build_b():
    import concourse.bass as bass
    from concourse import bacc, mybir
    import concourse.tile as tile

    fp32 = mybir.dt.float32
    fp32r = mybir.dt.float32r
    AF = mybir.ActivationFunctionType
    ALU = mybir.AluOpType

    nc = bacc.Bacc("TRN2", target_bir_lowering=False, debug=False, num_devices=8)

    woT = nc.declare_dram_parameter("woT", [D, D], fp32r, isOutput=False)
    ctxp = nc.declare_dram_parameter("ctxp", [H, DK, R], fp32r, isOutput=False)
    resid = nc.declare_dram_parameter("resid", [R, D], fp32, isOutput=False)
    y_o = nc.declare_dram_parameter("y_o", [R, D], fp32, isOutput=True)

    with tile.TileContext(nc) as tc:
        with (
            tc.tile_pool(name="persist", bufs=1) as persist,
            tc.tile_pool(name="wores", bufs=1) as wores_pool,
            tc.tile_pool(name="cxr", bufs=2) as cxr_pool,
            tc.tile_pool(name="rsp", bufs=2) as rs_pool,
            tc.tile_pool(name="outsb", bufs=2) as out_pool,
            tc.tile_pool(name="ln", bufs=4) as ln_pool,
            tc.tile_pool(name="psO", bufs=6, space="PSUM") as psO_pool,
        ):
            eps_sb = persist.tile([128, 1], fp32)
            nc.vector.memset(eps_sb, EPS)
            wo_sb = wores_pool.tile([128, H, D], fp32r)
            nc.sync.dma_start(
                out=wo_sb, in_=woT.rearrange("(c p) o -> p c o", p=128)
            )
            for rt in range(8):
                cxr = cxr_pool.tile([128, H, 128], fp32r, tag="cxr")
                nc.sync.dma_start(
                    out=cxr,
                    in_=ctxp[:, :, rt * 128 : (rt + 1) * 128].rearrange(
                        "h p r -> p h r"
                    ),
                )
                rs = rs_pool.tile([128, D], fp32, tag="rs")
                nc.sync.dma_start(out=rs, in_=resid[rt * 128 : (rt + 1) * 128, :])
                out_sb = out_pool.tile([128, D], fp32, tag="outsb")
                for og in range(4):
                    po = psO_pool.tile([128, 512], fp32, tag="po")
                    for hh in range(H):
                        nc.tensor.matmul(
                            po,
                            cxr[:, hh, :],
                            wo_sb[:, hh, og * 512 : (og + 1) * 512],
                            start=(hh == 0),
                            stop=(hh == H - 1),
                        )
                    nc.vector.tensor_add(
                        out_sb[:, og * 512 : (og + 1) * 512],
                        po,
                        rs[:, og * 512 : (og + 1) * 512],
                    )
                # LayerNorm over free dim (D)
                stats = ln_pool.tile(
                    [128, 4, nc.vector.BN_STATS_DIM], fp32, tag="stats"
                )
                for sg in range(4):
                    nc.vector.bn_stats(
                        out=stats[:, sg, :],
                        in_=out_sb[:, sg * 512 : (sg + 1) * 512],
                    )
                mv = ln_pool.tile([128, nc.vector.BN_AGGR_DIM], fp32, tag="mv")
                nc.vector.bn_aggr(out=mv, in_=stats)
                sd = ln_pool.tile([128, 1], fp32, tag="sd")
                nc.scalar.activation(
                    out=sd,
                    in_=mv[:, 1:2],
                    func=AF.Sqrt,
                    bias=eps_sb,
                    scale=1.0,
                )
                nc.vector.reciprocal(out=sd, in_=sd)
                nc.vector.tensor_scalar(
                    out=out_sb,
                    in0=out_sb,
                    scalar1=mv[:, 0:1],
                    scalar2=sd,
                    op0=ALU.subtract,
                    op1=ALU.mult,
                )
                nc.sync.dma_start(out=y_o[rt * 128 : (rt + 1) * 128, :], in_=out_sb)

    nc.compile()
    return nc


def _get_compiled():
    global _COMPILED
    if _COMPILED is None:
        _COMPILED = (_build_a(), _build_b())
    return _COMPILED


def _round_fp32r(x):
    from neuron_dtypes import static_cast_fp32_to_fp32r

    x = np.ascontiguousarray(x, dtype=np.float32)
    return np.asarray(static_cast_fp32_to_fp32r(x)).view(np.float32).reshape(x.shape)


def kernel(q, k, v, Wq, Wk, Wv, Wo, gamma, beta):
    global LAST_EXEC_NS
    from concourse.bass_utils import run_bass_kernel_spmd

    q = np.asarray(q, np.float32)
    k = np.asarray(k, np.float32)
    v = np.asarray(v, np.float32)

    nc_a, nc_b = _get_compiled()

    wqT = _round_fp32r(np.asarray(Wq, np.float32).T)
    wkT = _round_fp32r(np.asarray(Wk, np.float32).T)
    wvT = _round_fp32r(np.asarray(Wv, np.float32).T)
    woT = _round_fp32r(np.asarray(Wo, np.float32).T)
    ones1 = np.ones((1, 128), np.float32)

    in_maps = []
    for b in range(B):
        qT = _round_fp32r(q[b].T)
        kT = _round_fp32r(k[b].T)
        vT = _round_fp32r(v[b].T)
        for hf in range(2):
            in_maps.append(
                {
                    "qt": np.ascontiguousarray(qT[:, hf * R : (hf + 1) * R]),
                    "kt": kT,
                    "vt": vT,
                    "wqT": wqT,
                    "wkT": wkT,
                    "wvT": wvT,
                    "ones1": ones1,
                }
            )

    res_a = run_bass_kernel_spmd(nc_a, in_maps, list(range(8)), trace=TRACE)

    # Permute context slices across cores: output batch b' at feature slot h'
    # uses (head 4*b' + h'//4, batch h'%4), query half unchanged.
    in_maps_b = []
    for bp in range(B):
        for hf in range(2):
            ctxp = np.empty((H, DK, R), np.float32)
            for hp in range(H):
                src = res_a.results[(hp % 4) * 2 + hf]["ctx_s"]
                ctxp[hp] = src[4 * bp + hp // 4]
            in_maps_b.append(
                {
                    "woT": woT,
                    "ctxp": ctxp,
                    "resid": np.ascontiguousarray(q[bp, hf * R : (hf + 1) * R, :]),
                }
            )

    res_b = run_bass_kernel_spmd(nc_b, in_maps_b, list(range(8)), trace=TRACE)
    if res_a.exec_time_ns is not None and res_b.exec_time_ns is not None:
        LAST_EXEC_NS = res_a.exec_time_ns + res_b.exec_time_ns
    else:
        LAST_EXEC_NS = None

    attn = np.empty((H * B, S, S), np.float32)
    out = np.empty((B, S, D), np.float32)
    for b in range(B):
        for hf in range(2):
            core = b * 2 + hf
            out[b, hf * R : (hf + 1) * R, :] = res_b.results[core]["y_o"]
            for h in range(H):
                attn[h * B + b, hf * R : (hf + 1) * R, :] = res_a.results[core][
                    "attn_o"
                ][h]

    gamma = np.asarray(gamma, np.float32)
    beta = np.asarray(beta, np.float32)
    if not (np.all(gamma == 1.0) and np.all(beta == 0.0)):
        out = out * gamma + beta

    return out, attn
